# revision 8
# baseline (speedup 1.0000x reference)
"""Trainium2 Bass kernel for nn_Attention_43516608643501.

Cross-attention: Q = out_d [T,B,H]; K = V = sum of fwd/bwd halves of out_e
-> [S,B,H]; scores = Q @ K^T per batch (contraction over H, no scaling);
softmax over the source dim S; context = P @ V -> output [T,B,H].

Sharding: data-parallel over batch (dim 1): 2 batches per core x 8 cores,
no cross-core communication.

v2: ALL transposes ride the DMA crossbar (dma_start_transpose), not the
PE. oe/od are summed/cast to fp16 on DVE, round-tripped through a DRAM
scratch, and transposed back into [h, s]/[h, t] layout by the xbar
(16x128 source tiles, fp16). The softmax probabilities P [t, s] are
xbar-transposed SBUF->SBUF into [s, t] weight chunks for the P^T @ V
matmul, issued on the Activation queue so the SP queue stays free for
loads/stores. The PE therefore runs nothing but the 2 x 16 x 32 fp16
matmuls (512-wide moving streams).

The t-tile pipeline is depth-2 (stage2 trails stage1 by two tiles) so
the exp -> xbar -> mm2 chain (~5.5 us) hides under two PE tile-slots
(~6.8 us), and it is flattened across the two batches with batch 1's
prep DMAs interleaved into batch 0's main loop.

Numerics: fp16 matmuls (4.9e-4 rounding; scores carry no 1/sqrt(H)
scale, so softmax near-ties amplify score error by exp()). Per-row max
subtraction on the free dim keeps exp args <= 0.
"""

import numpy as np
from contextlib import ExitStack

S, T, B, H = 2048, 2048, 16, 512
NCORES = 8
BLOC = B // NCORES  # batches per core
P128 = 128
NS = S // P128  # 16 s-tiles
NT = T // P128  # 16 t-tiles
NH = H // P128  # 4 h-chunks of the contraction
SC = 512  # s-chunk width (scores tile columns)
NSC = S // SC  # 4 s-chunks per t-tile
GRP = 4  # s/t tiles per xbar group (512 rows)

_cached_nc = None


def _build():
    import concourse.bacc as bacc
    import concourse.tile as tile
    from concourse import mybir

    f32 = mybir.dt.float32
    f16 = mybir.dt.float16

    nc = bacc.Bacc(None, target_bir_lowering=False)
    d_oe = nc.dram_tensor("out_e", [S, BLOC, 2 * H], f32, kind="ExternalInput")
    d_od = nc.dram_tensor("out_d", [T, BLOC, H], f32, kind="ExternalInput")
    d_out = nc.dram_tensor("out", [T, BLOC, H], f32, kind="ExternalOutput")

    with ExitStack() as ctx:
        tc = ctx.enter_context(tile.TileContext(nc))
        loads = ctx.enter_context(tc.tile_pool(name="loads", bufs=6))
        persist = ctx.enter_context(tc.tile_pool(name="persist", bufs=2))
        ptile = ctx.enter_context(tc.tile_pool(name="ptile", bufs=3))
        ptile2 = ctx.enter_context(tc.tile_pool(name="ptile2", bufs=4))
        outs = ctx.enter_context(tc.tile_pool(name="outs", bufs=3))
        small = ctx.enter_context(tc.tile_pool(name="small", bufs=5))
        dscratch = ctx.enter_context(
            tc.tile_pool(name="dscratch", bufs=2, space="DRAM")
        )
        pdram = ctx.enter_context(tc.tile_pool(name="pdram", bufs=3, space="DRAM"))
        # PSUM: 8 banks = ps_s (4, freed chunk-by-chunk by exp) + ps_c (2x2)
        ps_s_pool = ctx.enter_context(
            tc.tile_pool(name="ps_s_pool", bufs=1, space="PSUM")
        )
        ps_cp = ctx.enter_context(tc.tile_pool(name="ps_cp", bufs=2, space="PSUM"))

        def make_batch(b):
            st = {
                "b": b,
                "oe_nat": [None] * NS,
                "oe16": dscratch.tile([S, H], f16, tag="oe16", name="oe16"),
                "od16": dscratch.tile([T, H], f16, tag="od16", name="od16"),
                "oeT_c": [
                    persist.tile([P128, NH, SC], f16, tag=f"oeT{g}", name=f"oeT{g}")
                    for g in range(NSC)
                ],
                "odT_g": [
                    persist.tile([P128, NH, SC], f16, tag=f"odT{g}", name=f"odT{g}")
                    for g in range(NT // GRP)
                ],
            }
            return st

        def prep_oe_group(bs, g):
            b = bs["b"]
            for k in range(GRP * g, GRP * g + GRP):
                raw = loads.tile([P128, 2 * H], f32, tag="raw", name="raw")
                nc.sync.dma_start(out=raw, in_=d_oe[k * P128:(k + 1) * P128, b, :])
                nat = persist.tile([P128, H], f16, tag=f"oenat{k}", name=f"oenat{k}")
                nc.vector.tensor_add(nat, raw[:, 0:H], raw[:, H:2 * H])
                bs["oe_nat"][k] = nat
                nc.sync.dma_start(out=bs["oe16"][k * P128:(k + 1) * P128, :], in_=nat)
            nc.sync.dma_start_transpose(
                bs["oeT_c"][g], bs["oe16"][g * SC:(g + 1) * SC, :]
            )

        def prep_od_group(bs, g):
            b = bs["b"]
            for k in range(GRP * g, GRP * g + GRP):
                odr = loads.tile([P128, H], f32, tag="odr", name="odr")
                nc.sync.dma_start(out=odr, in_=d_od[k * P128:(k + 1) * P128, b, :])
                odf = loads.tile([P128, H], f16, tag="odf", name="odf")
                nc.vector.tensor_copy(odf, odr)
                nc.sync.dma_start(out=bs["od16"][k * P128:(k + 1) * P128, :], in_=odf)
            nc.sync.dma_start_transpose(
                bs["odT_g"][g], bs["od16"][g * SC:(g + 1) * SC, :]
            )

        # ---- main: per t-tile of 128 query rows, software-pipelined:
        # stage 1 (tile i): scores matmuls + max + exp(P) + P xbar;
        # stage 2 (tile i-2): P^T @ V + 1/l scale + store. ----
        def stage1_begin(bs, tt):
            mx = small.tile([P128, NSC], f32, tag="mx", name="mx")
            return {"bs": bs, "tt": tt, "mx": mx, "ps_s": []}

        def stage1_chunk(st1, ci):
            bs, tt, mx = st1["bs"], st1["tt"], st1["mx"]
            pss = ps_s_pool.tile([P128, SC], f32, tag=f"ps_s{ci}", name=f"ps_s{ci}")
            wsel = bs["odT_g"][tt // GRP]
            toff = (tt % GRP) * P128
            for hc in range(NH):
                nc.tensor.matmul(
                    pss,
                    wsel[:, hc, toff:toff + P128],
                    bs["oeT_c"][ci][:, hc, :],
                    start=(hc == 0),
                    stop=(hc == NH - 1),
                )
            nc.vector.reduce_max(mx[:, ci:ci + 1], pss, axis=mybir.AxisListType.X)
            st1["ps_s"].append(pss)

        def stage1_finish(st1):
            bs, tt, mx, ps_s = st1["bs"], st1["tt"], st1["mx"], st1["ps_s"]
            neg_m = small.tile([P128, 1], f32, tag="neg_m", name="neg_m")
            m = small.tile([P128, 1], f32, tag="m", name="m")
            nc.vector.reduce_max(m, mx, axis=mybir.AxisListType.X)
            nc.vector.tensor_scalar_mul(neg_m, m, -1.0)

            lacc = small.tile([P128, NSC], f32, tag="lacc", name="lacc")
            pts = ptile.tile([P128, S], f16, tag="pts", name="pts")
            for ci in range(NSC):
                nc.scalar.activation(
                    pts[:, ci * SC:(ci + 1) * SC], ps_s[ci],
                    mybir.ActivationFunctionType.Exp,
                    bias=neg_m, scale=1.0,
                    accum_out=lacc[:, ci:ci + 1],
                )
            l = small.tile([P128, 1], f32, tag="l", name="l")
            nc.vector.reduce_sum(l, lacc, axis=mybir.AxisListType.X)
            linv = small.tile([P128, 1], f32, tag="linv", name="linv")
            nc.vector.reciprocal(linv, l)
            # P transpose rides the DMA crossbar through a DRAM bounce
            # (store + transpose back on the same queue): the direct
            # SBUF->SBUF xbar transpose loses a completion race against
            # the consuming LDWEIGHTS under DMA load.
            p16 = pdram.tile([P128, S], f16, tag="p16", name="p16")
            nc.sync.dma_start(out=p16[:, :], in_=pts)
            pT = ptile2.tile([P128, NS, P128], f16, tag="pT", name="pT")
            nc.sync.dma_start_transpose(pT, p16[:, :])
            return bs, tt, pT, linv

        def stage1(bs, tt):
            st1 = stage1_begin(bs, tt)
            for ci in range(NSC):
                stage1_chunk(st1, ci)
            return stage1_finish(st1)

        def stage2(state):
            bs, tt, pT, linv = state
            b = bs["b"]
            ps_c = ps_cp.tile([P128, H], f32, tag="ps_c", name="ps_c")
            for k in range(NS):
                nc.tensor.matmul(
                    ps_c,
                    pT[:, k, :],
                    bs["oe_nat"][k],
                    start=(k == 0), stop=(k == NS - 1),
                )
            ot = outs.tile([P128, H], f32, tag="ot", name="ot")
            nc.scalar.activation(
                ot, ps_c, mybir.ActivationFunctionType.Identity,
                bias=0.0, scale=linv,
            )
            nc.gpsimd.dma_start(
                out=d_out[tt * P128:(tt + 1) * P128, b, :], in_=ot
            )

        # ---- flattened two-batch pipeline ----
        b0 = make_batch(0)
        b1 = make_batch(1)

        # batch 0 startup: interleave the first t-tile's scores chunks into
        # the oe prep (chunk ci only needs oeT_c[ci] + odT_g[0]).
        prep_od_group(b0, 0)
        prep_oe_group(b0, 0)
        st1_0 = stage1_begin(b0, 0)
        for g in range(1, NSC):
            stage1_chunk(st1_0, g - 1)
            prep_oe_group(b0, g)
        prep_od_group(b0, 1)
        stage1_chunk(st1_0, NSC - 1)
        states = [stage1_finish(st1_0)]

        # prep of batch-b0 odT groups 2,3 one group ahead; batch-1 prep
        # interleaved into batch 0's tail.
        prep_at = {
            2: (b0, "od", 2), 5: (b0, "od", 3),
            8: (b1, "oe", 0), 9: (b1, "oe", 1), 10: (b1, "oe", 2),
            11: (b1, "oe", 3), 12: (b1, "od", 0), 13: (b1, "od", 1),
            18: (b1, "od", 2), 21: (b1, "od", 3),
        }
        for i in range(1, 2 * NT):
            bs, tt = (b0, i) if i < NT else (b1, i - NT)
            if i in prep_at:
                pbs, kind, g = prep_at[i]
                (prep_oe_group if kind == "oe" else prep_od_group)(pbs, g)
            states.append(stage1(bs, tt))
            if len(states) >= 4:
                stage2(states.pop(0))
        while states:
            stage2(states.pop(0))

    nc.finalize()
    return nc


def _ensure_devices():
    """Make sure the 8 NeuronCores are visible to jax.devices().

    The calling harness may have pinned jax to cpu (JAX_PLATFORMS=cpu is a
    common pin for running the jax reference); the Bass SPMD launcher uses
    jax.devices(), so re-point jax at the neuron platform if needed.
    """
    import os
    import jax

    try:
        devs = jax.devices()
    except Exception:
        devs = []
    if sum(1 for d in devs if d.platform != "cpu") >= NCORES:
        return
    for plats in ("axon,cpu", None):
        try:
            if plats is None:
                os.environ.pop("JAX_PLATFORMS", None)
            else:
                os.environ["JAX_PLATFORMS"] = plats
            jax.config.update("jax_platforms", plats)
            from jax.extend.backend import clear_backends

            clear_backends()
            devs = jax.devices()
            if sum(1 for d in devs if d.platform != "cpu") >= NCORES:
                return
        except Exception:
            continue


def kernel(in_e=None, out_e=None, out_d=None, **kwargs):
    global _cached_nc
    from concourse.bass_utils import run_bass_kernel_spmd

    _ensure_devices()

    out_e = np.asarray(out_e, dtype=np.float32)
    out_d = np.asarray(out_d, dtype=np.float32)
    if _cached_nc is None:
        _cached_nc = _build()
    in_maps = []
    for c in range(NCORES):
        bsl = slice(c * BLOC, (c + 1) * BLOC)
        in_maps.append({
            "out_e": np.ascontiguousarray(out_e[:, bsl, :]),
            "out_d": np.ascontiguousarray(out_d[:, bsl, :]),
        })
    res = run_bass_kernel_spmd(_cached_nc, in_maps, list(range(NCORES)))
    return np.concatenate([res.results[c]["out"] for c in range(NCORES)], axis=1)
